# revision 1
# baseline (speedup 1.0000x reference)
"""Trainium2 Bass kernel for a 2-layer GAT + global mean pool + linear head.

Math (matches PyG GATConv, eval mode, single head, add_self_loops=True):
  h   = x @ W
  e_k = lrelu(ss[src_k] + sd[dst_k]),  ss = h@a_src, sd = h@a_dst
  alpha = softmax over incoming edges of each dst (self-loop included)
  out[d] = sum_k alpha_k h[src_k] + b
Two GAT layers (512->128, 128->64) with ReLU, then per-graph mean pool
over `batch` and a final [64,2] linear.

Strategy (8 NeuronCores, full inputs in / full output out):
  * Destination nodes sharded across cores (2500/core), sources arbitrary.
  * Each core computes h for its shard, assembles a gather table row
    [h | ss | pad] per node, AllGathers the table into every core's HBM.
  * Edges grouped per destination into fixed "slots" (padded with a
    sentinel table row that contributes ~0 to the softmax) and laid out
    destination-per-partition. Slot source rows are fetched with one
    indirect DMA per (block, sub-block).
  * Attention: ACT Lrelu (bias = per-partition sd) + ACT Exp with
    accum_out = softmax denominator. Aggregation: DVE broadcast-multiply
    + strided tensor_reduce. Dense matmuls/transposes/pooling on PE.
  * Per-graph pooling one-hots (with 1/count folded in) are host-built;
    partial pooled features are AllReduced, final linear on every core.

All graph-structure preprocessing (degree sort, slot layout, index
remapping) is host-side numpy on the kernel inputs; the device only sees
dense arrays.
"""

import math
import numpy as np

import concourse.bass as bass
import concourse.bacc as bacc
import concourse.mybir as mybir
from concourse.tile import TileContext
from concourse.masks import make_identity
from concourse.bass_utils import run_bass_kernel_spmd

F32 = mybir.dt.float32
BF16 = mybir.dt.bfloat16
I32 = mybir.dt.int32
AF = mybir.ActivationFunctionType
ALU = mybir.AluOpType

NEG_SLOPE = 0.2
SENT_SS = -60.0  # sentinel row score: exp(lrelu(-60+sd)) ~ e^-10 -> harmless


def full_cfg():
    return dict(N=20000, IND=512, HID=128, HID2=64, OUT=2, G=16, NCORES=8,
                LCAP=48)


# ----------------------------------------------------------------------------
# Host-side preprocessing
# ----------------------------------------------------------------------------

def preprocess(x, edge_index, batch, W1, a1_src, a1_dst, b1,
               W2, a2_src, a2_dst, b2, Wl, bl, cfg):
    N, IND, HID, HID2, OUT, G, NC = (cfg[k] for k in
                                     ("N", "IND", "HID", "HID2", "OUT", "G",
                                      "NCORES"))
    PC = math.ceil(N / NC)            # real dests per core
    PB = math.ceil(PC / 128)          # dest blocks per core
    PCP = PB * 128                    # padded dests per core
    TR = NC * PCP + 1                 # table rows (+1 sentinel)
    SENT = TR - 1
    R1 = HID + 1       # bf16 table row: [h | ss]
    R2 = HID2 + 1

    x = np.asarray(x, np.float32)
    batch = np.asarray(batch, np.int64)
    src = np.asarray(edge_index[0], np.int64)
    dst = np.asarray(edge_index[1], np.int64)
    # self loops
    loop = np.arange(N, dtype=np.int64)
    src = np.concatenate([src, loop])
    dst = np.concatenate([dst, loop])

    counts = np.bincount(batch, minlength=G).astype(np.float64)

    # per-core degree-sorted permutations and global row ids
    row_of = np.empty(N, np.int64)       # global node -> table row
    orders = []
    degs_sorted = np.zeros((NC, PCP), np.int64)
    core_of_dst = np.minimum(dst // PC, NC - 1)
    for k in range(NC):
        lo, hi = k * PC, min((k + 1) * PC, N)
        nk = hi - lo
        mask = (dst >= lo) & (dst < hi)
        deg = np.bincount(dst[mask] - lo, minlength=nk)
        order = np.argsort(-deg, kind="stable")        # local rank -> local id
        inv = np.empty(nk, np.int64)
        inv[order] = np.arange(nk)
        row_of[lo:hi] = k * PCP + inv
        orders.append(order)
        degs_sorted[k, :nk] = deg[order]

    # global per-block slot counts (identical program on every core)
    Ls = []
    for b in range(PB):
        Lb = int(degs_sorted[:, b * 128:(b + 1) * 128].max())
        Ls.append(max(Lb, 1))
    S = int(np.sum(Ls))
    offs = np.concatenate([[0], np.cumsum(Ls)]).astype(np.int64)

    # shared (replicated) weight uploads
    KB = IND // 128
    W1u = np.ascontiguousarray(W1.astype(np.float32).reshape(KB, 128, HID))
    W2u = np.ascontiguousarray(W2.astype(np.float32))
    a1s = np.tile(np.asarray(a1_src, np.float32)[None, :], (128, 1))
    a1d = np.tile(np.asarray(a1_dst, np.float32)[None, :], (128, 1))
    b1r = np.tile(np.asarray(b1, np.float32)[None, :], (128, 1))
    a2s = np.tile(np.asarray(a2_src, np.float32)[None, :], (128, 1))
    a2d = np.tile(np.asarray(a2_dst, np.float32)[None, :], (128, 1))
    b2r = np.tile(np.asarray(b2, np.float32)[None, :], (128, 1))
    WlBl = np.concatenate([np.asarray(Wl, np.float32),
                           np.asarray(bl, np.float32)[None, :]], axis=0)
    import ml_dtypes
    sent1 = np.zeros((1, R1), ml_dtypes.bfloat16)
    sent1[0, HID] = SENT_SS
    sent2 = np.zeros((1, R2), ml_dtypes.bfloat16)
    sent2[0, HID2] = SENT_SS

    in_maps = []
    for k in range(NC):
        lo, hi = k * PC, min((k + 1) * PC, N)
        nk = hi - lo
        order = orders[k]

        # xT: [KB, 128, PCP] (feature-major columns in local-rank order)
        xs = np.zeros((PCP, IND), np.float32)
        xs[:nk] = x[lo:hi][order]
        xT = np.ascontiguousarray(xs.T.reshape(KB, 128, PCP))

        # slot indices [128, S] -> table rows, sentinel padded
        sidx = np.full((128, S), SENT, np.int64)
        mask = (dst >= lo) & (dst < hi)
        es, ed = src[mask], dst[mask] - lo
        o = np.argsort(ed, kind="stable")
        es, ed = es[o], ed[o]
        deg = np.bincount(ed, minlength=nk)
        start = np.concatenate([[0], np.cumsum(deg)[:-1]])
        j = np.arange(len(ed)) - start[ed]            # slot within dest
        inv = np.empty(nk, np.int64)
        inv[order] = np.arange(nk)
        r = inv[ed]                                   # dest rank
        bb, pp = r // 128, r % 128
        col = offs[bb] + j
        sidx[pp, col] = row_of[es]
        idxw = sidx.astype(np.int32)

        # pooling one-hot with 1/count folded, zero rows for pad dests
        P = np.zeros((128, PB * G), np.float32)
        bg = batch[lo:hi][order]                      # graph id per rank
        rr = np.arange(nk)
        P[rr % 128, (rr // 128) * G + bg] = 1.0 / np.maximum(counts[bg], 1.0)

        in_maps.append(dict(
            xT=xT, W1u=W1u, W2u=W2u, a1s=a1s, a1d=a1d, b1r=b1r,
            a2s=a2s, a2d=a2d, b2r=b2r, WlBl=WlBl.astype(np.float32),
            Pp=P, sidx=idxw, sent1=sent1, sent2=sent2,
        ))

    meta = dict(PC=PC, PB=PB, PCP=PCP, TR=TR, R1=R1, R2=R2, KB=KB, S=S,
                Ls=Ls, offs=offs)
    return in_maps, meta


# ----------------------------------------------------------------------------
# Device program
# ----------------------------------------------------------------------------

def build_program(cfg, meta, debug_outs=False):
    N, IND, HID, HID2, OUT, G, NC, LCAP = (cfg[k] for k in
                                           ("N", "IND", "HID", "HID2", "OUT",
                                            "G", "NCORES", "LCAP"))
    PB, PCP, TR, R1, R2, KB, S = (meta[k] for k in
                                  ("PB", "PCP", "TR", "R1", "R2", "KB", "S"))

    Ls, offs = meta["Ls"], meta["offs"]

    nc = bacc.Bacc("TRN2", target_bir_lowering=False, debug=False,
                   num_devices=NC)

    xT_d = nc.declare_dram_parameter("xT", [KB, 128, PCP], F32, False)
    W1_d = nc.declare_dram_parameter("W1u", [KB, 128, HID], F32, False)
    W2_d = nc.declare_dram_parameter("W2u", [HID, HID2], F32, False)
    a1s_d = nc.declare_dram_parameter("a1s", [128, HID], F32, False)
    a1d_d = nc.declare_dram_parameter("a1d", [128, HID], F32, False)
    b1r_d = nc.declare_dram_parameter("b1r", [128, HID], F32, False)
    a2s_d = nc.declare_dram_parameter("a2s", [128, HID2], F32, False)
    a2d_d = nc.declare_dram_parameter("a2d", [128, HID2], F32, False)
    b2r_d = nc.declare_dram_parameter("b2r", [128, HID2], F32, False)
    Wl_d = nc.declare_dram_parameter("WlBl", [HID2 + 1, OUT], F32, False)
    Pp_d = nc.declare_dram_parameter("Pp", [128, PB * G], F32, False)
    sidx_d = nc.declare_dram_parameter("sidx", [128, S], I32, False)
    sent1_d = nc.declare_dram_parameter("sent1", [1, R1], BF16, False)
    sent2_d = nc.declare_dram_parameter("sent2", [1, R2], BF16, False)
    out_d = nc.declare_dram_parameter("out", [G, OUT], F32, True)
    if debug_outs:
        dbg_t1 = nc.declare_dram_parameter("dbg_t1", [TR, R1], BF16, True)
        dbg_sd1 = nc.declare_dram_parameter("dbg_sd1", [128, PB], F32, True)
        dbg_r2 = nc.declare_dram_parameter("dbg_r2", [PB * 128, HID2], F32,
                                           True)
        dbg_r1 = nc.declare_dram_parameter("dbg_r1", [PB * 128, HID], F32,
                                           True)
        dbg_g = nc.declare_dram_parameter("dbg_g", [128, LCAP * R1], BF16,
                                          True)
        dbg_den = nc.declare_dram_parameter("dbg_den", [128, PB], F32, True)

    shared = dict(addr_space="Shared") if NC > 4 else {}
    T1shard = nc.dram_tensor("T1shard", [PCP, R1], BF16)
    T1full = nc.dram_tensor("T1full", [TR, R1], BF16, **shared)
    T2shard = nc.dram_tensor("T2shard", [PCP, R2], BF16)
    T2full = nc.dram_tensor("T2full", [TR, R2], BF16, **shared)
    poolin = nc.dram_tensor("poolin", [G, HID2], F32)
    poolout = nc.dram_tensor("poolout", [G, HID2], F32, **shared)

    groups = [list(range(NC))]

    def subs_of(L):
        return [(s, min(LCAP, L - s)) for s in range(0, L, LCAP)]

    with TileContext(nc) as tc:
        with (
            tc.tile_pool(name="const", bufs=1) as cp,
            tc.tile_pool(name="work", bufs=3) as wp,
            tc.tile_pool(name="gath", bufs=2) as gp,
            tc.tile_pool(name="wtp", bufs=2) as wtp,
            tc.tile_pool(name="psA", bufs=2, space="PSUM") as psA,
            tc.tile_pool(name="psB", bufs=2, space="PSUM") as psB,
            tc.tile_pool(name="psP", bufs=1, space="PSUM") as psP,
        ):
            # ---------------- constants to SBUF ----------------
            W1_sb = cp.tile([128, KB * HID], F32, tag="w1")
            W1v = W1_sb[:].rearrange("p (k h) -> p k h", h=HID)
            nc.sync.dma_start(out=W1v, in_=W1_d[:].rearrange("k p h -> p k h"))
            W2_sb = cp.tile([HID, HID2], F32, tag="w2")
            nc.sync.dma_start(out=W2_sb[:], in_=W2_d[:])
            a1s_sb = cp.tile([128, HID], F32, tag="a1s")
            nc.sync.dma_start(out=a1s_sb[:], in_=a1s_d[:])
            a1d_sb = cp.tile([128, HID], F32, tag="a1d")
            nc.sync.dma_start(out=a1d_sb[:], in_=a1d_d[:])
            b1r_sb = cp.tile([128, HID], F32, tag="b1r")
            nc.sync.dma_start(out=b1r_sb[:], in_=b1r_d[:])
            a2s_sb = cp.tile([128, HID2], F32, tag="a2s")
            nc.sync.dma_start(out=a2s_sb[:], in_=a2s_d[:])
            a2d_sb = cp.tile([128, HID2], F32, tag="a2d")
            nc.sync.dma_start(out=a2d_sb[:], in_=a2d_d[:])
            b2r_sb = cp.tile([128, HID2], F32, tag="b2r")
            nc.sync.dma_start(out=b2r_sb[:], in_=b2r_d[:])
            Wl_sb = cp.tile([HID2 + 1, OUT], F32, tag="wl")
            nc.sync.dma_start(out=Wl_sb[:], in_=Wl_d[:])
            P_sb = cp.tile([128, PB * G], F32, tag="pp")
            nc.sync.dma_start(out=P_sb[:], in_=Pp_d[:])
            sidx_sb = cp.tile([128, S], I32, tag="sidx")
            nc.sync.dma_start(out=sidx_sb[:], in_=sidx_d[:])
            ident = cp.tile([128, 128], F32, tag="id")
            make_identity(nc, ident[:])

            T1sb = cp.tile([128, PB * R1], BF16, tag="t1")
            nc.vector.memset(T1sb[:], 0.0)
            T2sb = cp.tile([128, PB * R2], BF16, tag="t2")
            nc.vector.memset(T2sb[:], 0.0)
            sd1 = cp.tile([128, PB], F32, tag="sd1")
            sd2 = cp.tile([128, PB], F32, tag="sd2")
            ss1f = cp.tile([128, PB], F32, tag="ss1f")
            ss2f = cp.tile([128, PB], F32, tag="ss2f")

            # ---------------- phase A: h1 / scores / T1 ----------------
            with tc.tile_pool(name="xpool", bufs=3) as xp:
                for c in range(PB):
                    xc = xp.tile([128, KB * 128], F32, tag="xc")
                    xcv = xc[:].rearrange("p (k n) -> p k n", n=128)
                    nc.sync.dma_start(
                        out=xcv,
                        in_=xT_d[:, :, c * 128:(c + 1) * 128]
                        .rearrange("k p n -> p k n"))
                    ph = psA.tile([128, HID], F32, tag="ph")
                    for kb in range(KB):
                        nc.tensor.matmul(
                            ph[:],
                            lhsT=xc[:, kb * 128:(kb + 1) * 128],
                            rhs=W1_sb[:, kb * HID:(kb + 1) * HID],
                            start=(kb == 0), stop=(kb == KB - 1),
                        )
                    nc.vector.tensor_copy(
                        T1sb[:, c * R1:c * R1 + HID], ph[:])
                    tmp = wp.tile([128, HID], F32, tag="sc")
                    nc.vector.tensor_tensor(
                        out=tmp[:], in0=ph[:], in1=a1s_sb[:], op=ALU.mult)
                    nc.vector.tensor_reduce(
                        out=ss1f[:, c:c + 1], in_=tmp[:],
                        axis=mybir.AxisListType.X, op=ALU.add)
                    nc.vector.tensor_copy(
                        T1sb[:].rearrange("p (c w) -> p c w", w=R1)
                        [:, c:c + 1, HID], ss1f[:, c:c + 1])
                    tmp2 = wp.tile([128, HID], F32, tag="sc")
                    nc.vector.tensor_tensor(
                        out=tmp2[:], in0=ph[:], in1=a1d_sb[:], op=ALU.mult)
                    nc.vector.tensor_reduce(
                        out=sd1[:, c:c + 1], in_=tmp2[:],
                        axis=mybir.AxisListType.X, op=ALU.add)
                    nc.sync.dma_start(
                        out=T1shard[c * 128:(c + 1) * 128, :],
                        in_=T1sb[:, c * R1:(c + 1) * R1])
            nc.sync.dma_start(out=T1full[TR - 1:TR, :], in_=sent1_d[:])
            nc.gpsimd.collective_compute(
                "AllGather", ALU.bypass, replica_groups=groups,
                ins=[T1shard[:]], outs=[T1full[0:TR - 1, :]])

            # ---------------- phase B: GAT layer 1 ----------------
            for b in range(PB):
                L = Ls[b]
                o_t = wp.tile([128, HID], F32, tag="o1")
                den = wp.tile([128, 1], F32, tag="den")
                for si, (s0, Lc) in enumerate(subs_of(L)):
                    Gt = gp.tile([128, Lc * R1], BF16, tag="g1")
                    Gv = Gt[:].rearrange("p (l w) -> p l w", w=R1)
                    for j in range(Lc):
                        gc = gp.tile([128, R1], BF16, tag="gc")
                        nc.gpsimd.indirect_dma_start(
                            out=gc[:], out_offset=None,
                            in_=T1full[:],
                            in_offset=bass.IndirectOffsetOnAxis(
                                ap=sidx_sb[:, offs[b] + s0 + j:
                                           offs[b] + s0 + j + 1],
                                axis=0))
                        nc.vector.tensor_copy(Gv[:, j:j + 1, :], gc[:])
                    if debug_outs and b == 0 and si == 0:
                        nc.sync.dma_start(out=dbg_g[:, :Lc * R1], in_=Gt[:])
                    t_t = wp.tile([128, Lc], F32, tag="tpre")
                    nc.vector.tensor_scalar(
                        out=t_t[:], in0=Gv[:, :, HID],
                        scalar1=sd1[:, b:b + 1], scalar2=None, op0=ALU.add)
                    u_t = wp.tile([128, Lc], F32, tag="upre")
                    nc.vector.tensor_scalar(
                        out=u_t[:], in0=t_t[:], scalar1=NEG_SLOPE,
                        scalar2=None, op0=ALU.mult)
                    wl_t = wp.tile([128, Lc], F32, tag="wl1")
                    nc.vector.tensor_tensor(
                        out=wl_t[:], in0=t_t[:], in1=u_t[:], op=ALU.max)
                    wex = wp.tile([128, Lc], F32, tag="we1")
                    dsub = wp.tile([128, 1], F32, tag="dsub")
                    nc.scalar.activation(
                        wex[:], wl_t[:], AF.Exp, accum_out=dsub[:])
                    wt = wtp.tile([128, Lc * HID], F32, tag="wt")
                    nc.vector.tensor_tensor(
                        out=wt[:], in0=Gv[:, :, 0:HID],
                        in1=wex[:, :, None].to_broadcast([128, Lc, HID]),
                        op=ALU.mult)
                    if si == 0:
                        nc.vector.tensor_copy(den[:], dsub[:])
                        nc.vector.tensor_reduce(
                            out=o_t[:],
                            in_=wt[:].rearrange("p (l f) -> p f l", f=HID),
                            axis=mybir.AxisListType.X, op=ALU.add)
                    else:
                        nc.vector.tensor_tensor(
                            out=den[:], in0=den[:], in1=dsub[:], op=ALU.add)
                        o_s = wp.tile([128, HID], F32, tag="o1s")
                        nc.vector.tensor_reduce(
                            out=o_s[:],
                            in_=wt[:].rearrange("p (l f) -> p f l", f=HID),
                            axis=mybir.AxisListType.X, op=ALU.add)
                        nc.vector.tensor_tensor(
                            out=o_t[:], in0=o_t[:], in1=o_s[:], op=ALU.add)
                rec = wp.tile([128, 1], F32, tag="rec")
                nc.vector.reciprocal(rec[:], den[:])
                ob = wp.tile([128, HID], F32, tag="ob")
                nc.vector.scalar_tensor_tensor(
                    out=ob[:], in0=o_t[:], scalar=rec[:], in1=b1r_sb[:],
                    op0=ALU.mult, op1=ALU.add)
                r1 = wp.tile([128, HID], F32, tag="r1")
                nc.scalar.activation(r1[:], ob[:], AF.Relu)
                if debug_outs:
                    nc.sync.dma_start(
                        out=dbg_r1[b * 128:(b + 1) * 128, :], in_=r1[:])
                    nc.sync.dma_start(out=dbg_den[:, b:b + 1], in_=den[:])
                # transpose -> h2 = r1 @ W2, plus layer-2 scores
                pT = psB.tile([128, HID], F32, tag="tr")
                nc.tensor.transpose(pT[:], r1[:], identity=ident[:])
                r1T = wp.tile([128, HID], F32, tag="r1T")
                nc.vector.tensor_copy(r1T[:], pT[:])
                ph2 = psB.tile([128, HID2], F32, tag="tr")
                nc.tensor.matmul(ph2[:], lhsT=r1T[:], rhs=W2_sb[:],
                                 start=True, stop=True)
                nc.vector.tensor_copy(T2sb[:, b * R2:b * R2 + HID2], ph2[:])
                tmp = wp.tile([128, HID2], F32, tag="sc2")
                nc.vector.tensor_tensor(
                    out=tmp[:], in0=ph2[:], in1=a2s_sb[:], op=ALU.mult)
                nc.vector.tensor_reduce(
                    out=ss2f[:, b:b + 1], in_=tmp[:],
                    axis=mybir.AxisListType.X, op=ALU.add)
                nc.vector.tensor_copy(
                    T2sb[:].rearrange("p (c w) -> p c w", w=R2)
                    [:, b:b + 1, HID2], ss2f[:, b:b + 1])
                tmp2 = wp.tile([128, HID2], F32, tag="sc2")
                nc.vector.tensor_tensor(
                    out=tmp2[:], in0=ph2[:], in1=a2d_sb[:], op=ALU.mult)
                nc.vector.tensor_reduce(
                    out=sd2[:, b:b + 1], in_=tmp2[:],
                    axis=mybir.AxisListType.X, op=ALU.add)
                nc.sync.dma_start(
                    out=T2shard[b * 128:(b + 1) * 128, :],
                    in_=T2sb[:, b * R2:(b + 1) * R2])


            nc.sync.dma_start(out=T2full[TR - 1:TR, :], in_=sent2_d[:])
            nc.gpsimd.collective_compute(
                "AllGather", ALU.bypass, replica_groups=groups,
                ins=[T2shard[:]], outs=[T2full[0:TR - 1, :]])

            # ---------------- phase C: GAT layer 2 + pooling ----------------
            pool_ps = psP.tile([G, HID2], F32, tag="pool")
            for b in range(PB):
                L = Ls[b]
                o_t = wp.tile([128, HID2], F32, tag="o2")
                den = wp.tile([128, 1], F32, tag="den")
                for si, (s0, Lc) in enumerate(subs_of(L)):
                    Gt = gp.tile([128, Lc * R2], BF16, tag="g1")
                    Gv = Gt[:].rearrange("p (l w) -> p l w", w=R2)
                    for j in range(Lc):
                        gc = gp.tile([128, R2], BF16, tag="gc")
                        nc.gpsimd.indirect_dma_start(
                            out=gc[:], out_offset=None,
                            in_=T2full[:],
                            in_offset=bass.IndirectOffsetOnAxis(
                                ap=sidx_sb[:, offs[b] + s0 + j:
                                           offs[b] + s0 + j + 1],
                                axis=0))
                        nc.vector.tensor_copy(Gv[:, j:j + 1, :], gc[:])
                    t_t = wp.tile([128, Lc], F32, tag="tpre")
                    nc.vector.tensor_scalar(
                        out=t_t[:], in0=Gv[:, :, HID2],
                        scalar1=sd2[:, b:b + 1], scalar2=None, op0=ALU.add)
                    u_t = wp.tile([128, Lc], F32, tag="upre")
                    nc.vector.tensor_scalar(
                        out=u_t[:], in0=t_t[:], scalar1=NEG_SLOPE,
                        scalar2=None, op0=ALU.mult)
                    wl_t = wp.tile([128, Lc], F32, tag="wl1")
                    nc.vector.tensor_tensor(
                        out=wl_t[:], in0=t_t[:], in1=u_t[:], op=ALU.max)
                    wex = wp.tile([128, Lc], F32, tag="we1")
                    dsub = wp.tile([128, 1], F32, tag="dsub")
                    nc.scalar.activation(
                        wex[:], wl_t[:], AF.Exp, accum_out=dsub[:])
                    wt = wtp.tile([128, Lc * HID2], F32, tag="wt")
                    nc.vector.tensor_tensor(
                        out=wt[:], in0=Gv[:, :, 0:HID2],
                        in1=wex[:, :, None].to_broadcast([128, Lc, HID2]),
                        op=ALU.mult)
                    if si == 0:
                        nc.vector.tensor_copy(den[:], dsub[:])
                        nc.vector.tensor_reduce(
                            out=o_t[:],
                            in_=wt[:].rearrange("p (l f) -> p f l", f=HID2),
                            axis=mybir.AxisListType.X, op=ALU.add)
                    else:
                        nc.vector.tensor_tensor(
                            out=den[:], in0=den[:], in1=dsub[:], op=ALU.add)
                        o_s = wp.tile([128, HID2], F32, tag="o2s")
                        nc.vector.tensor_reduce(
                            out=o_s[:],
                            in_=wt[:].rearrange("p (l f) -> p f l", f=HID2),
                            axis=mybir.AxisListType.X, op=ALU.add)
                        nc.vector.tensor_tensor(
                            out=o_t[:], in0=o_t[:], in1=o_s[:], op=ALU.add)
                rec = wp.tile([128, 1], F32, tag="rec")
                nc.vector.reciprocal(rec[:], den[:])
                ob = wp.tile([128, HID2], F32, tag="ob2")
                nc.vector.scalar_tensor_tensor(
                    out=ob[:], in0=o_t[:], scalar=rec[:], in1=b2r_sb[:],
                    op0=ALU.mult, op1=ALU.add)
                r2 = wp.tile([128, HID2], F32, tag="r2")
                nc.scalar.activation(r2[:], ob[:], AF.Relu)
                nc.tensor.matmul(
                    pool_ps[:], lhsT=P_sb[:, b * G:(b + 1) * G], rhs=r2[:],
                    start=(b == 0), stop=(b == PB - 1))
                if debug_outs:
                    nc.sync.dma_start(
                        out=dbg_r2[b * 128:(b + 1) * 128, :], in_=r2[:])

            if debug_outs:
                nc.sync.dma_start(out=dbg_t1[:], in_=T1full[:])
                nc.sync.dma_start(out=dbg_sd1[:], in_=sd1[:])
            pooled = wp.tile([G, HID2], F32, tag="pool")
            nc.vector.tensor_copy(pooled[:], pool_ps[:])
            nc.sync.dma_start(out=poolin[:], in_=pooled[:])
            nc.gpsimd.collective_compute(
                "AllReduce", ALU.add, replica_groups=groups,
                ins=[poolin[:]], outs=[poolout[:]])
            pooled_r = wp.tile([G, HID2], F32, tag="poolr")
            nc.sync.dma_start(out=pooled_r[:], in_=poolout[:])
            pTf = psB.tile([HID2, G], F32, tag="tr")
            nc.tensor.transpose(pTf[:], pooled_r[:], identity=ident[:G, :G])
            fin = wp.tile([HID2 + 1, G], F32, tag="fin")
            nc.vector.tensor_copy(fin[:HID2, :], pTf[:])
            nc.vector.memset(fin[HID2:HID2 + 1, :], 1.0)
            out_ps = psB.tile([G, OUT], F32, tag="tr")
            nc.tensor.matmul(out_ps[:], lhsT=fin[:], rhs=Wl_sb[:],
                             start=True, stop=True)
            out_sb = wp.tile([G, OUT], F32, tag="outsb")
            nc.vector.tensor_copy(out_sb[:], out_ps[:])
            nc.sync.dma_start(out=out_d[:], in_=out_sb[:])

    nc.compile()
    return nc


# ----------------------------------------------------------------------------
# Entry point
# ----------------------------------------------------------------------------

LAST_RESULTS = None


def kernel(**inputs):
    global LAST_RESULTS
    cfg = full_cfg()
    in_maps, meta = preprocess(cfg=cfg, **inputs)
    nc = build_program(cfg, meta)
    res = run_bass_kernel_spmd(nc, in_maps, core_ids=list(range(cfg["NCORES"])))
    LAST_RESULTS = res
    return np.asarray(res.results[0]["out"], np.float32)



# revision 50
# speedup vs baseline: 1.0413x; 1.0413x over previous
"""Trainium2 Bass kernel for a 2-layer GAT + global mean pool + linear head.

Math (matches PyG GATConv, eval mode, single head, add_self_loops=True):
  h   = x @ W
  e_k = lrelu(ss[src_k] + sd[dst_k]),  ss = h@a_src, sd = h@a_dst
  alpha = softmax over incoming edges of each dst (self-loop included)
  out[d] = sum_k alpha_k h[src_k] + b
Two GAT layers (512->128, 128->64) with ReLU, then per-graph mean pool
over `batch` and a final [64,2] linear.

Strategy (8 NeuronCores, full inputs in / full output out):
  * Destination nodes sharded across cores (2500/core), sources arbitrary.
  * a_src/a_dst are folded into the weight matrices host-side
    (W1e = [W1 | W1@a1_src | W1@a1_dst]), so the phase-A matmul (bf16 on
    PE) directly produces [h | ss | sd] per node. Table rows are packed
    as bytes [h_fp8e4m3 | ss_bf16] (130B layer 1, 66B layer 2),
    AllGathered compactly (layer 2 in chunks overlapped with phase-B
    compute), then locally expanded to a 256B-pitch copy because
    dma_gather requires 256B-multiple rows.
  * Edges grouped per destination into fixed "slots" (padded with a
    sentinel table row whose score ~ -60 contributes ~0 to the softmax),
    destinations per-partition, degree-sorted so each 128-dest block has
    a small uniform slot count L. Source rows are fetched with
    gpsimd.dma_gather (true multi-index gather; ucode caps one
    instruction at 1024 indices, so each block issues ceil(L*128/1024)
    sub-gathers into slices of one tile). NOTE: indirect_dma_start with
    a multi-column offset AP does NOT do a per-element gather on real
    HW (it fetches consecutive rows from each partition's first index).
  * Attention on ACT: Prelu (bias = per-partition sd, alpha=0.2;
    Prelu shares the "exp_and_others" act table with Exp/Relu so no
    table thrashing) then Exp with accum_out = softmax denominator.
    Per-edge weighting is split ~63/37 between ACT (per-slot scale-copy)
    and DVE (one broadcast multiply) so both engines finish together;
    the segment sum is a strided tensor_reduce on DVE.
  * Per-graph pooling one-hots (with 1/count folded in) are host-built
    and applied on PE; each core returns its pooled partials and the
    host sums them and applies the [64,2] head (cheaper than an
    on-device AllReduce + replicated linear tail).
"""

import math
import numpy as np

import concourse.bass as bass
import concourse.bacc as bacc
import concourse.mybir as mybir
from concourse.tile import TileContext
from concourse.masks import make_identity
from concourse.bass_utils import run_bass_kernel_spmd

F32 = mybir.dt.float32
BF16 = mybir.dt.bfloat16
I32 = mybir.dt.int32
I16 = mybir.dt.int16
U8 = mybir.dt.uint8
FP8 = mybir.dt.float8e4
AF = mybir.ActivationFunctionType
ALU = mybir.AluOpType

NEG_SLOPE = 0.2
SENT_SS = -60.0  # sentinel row score: exp(lrelu(-60+sd)) ~ e^-12 -> harmless


def full_cfg():
    return dict(N=20000, IND=512, HID=128, HID2=64, OUT=2, G=16, NCORES=8,
                NCH1=2, NCH2=4, SSB=5)


# ----------------------------------------------------------------------------
# Host-side preprocessing
# ----------------------------------------------------------------------------

def preprocess(x, edge_index, batch, W1, a1_src, a1_dst, b1,
               W2, a2_src, a2_dst, b2, Wl, bl, cfg):
    N, IND, HID, HID2, OUT, G, NC = (cfg[k] for k in
                                     ("N", "IND", "HID", "HID2", "OUT", "G",
                                      "NCORES"))
    PC = math.ceil(N / NC)            # real dests per core
    PB = math.ceil(PC / 128)          # dest blocks per core
    PCP = PB * 128                    # padded dests per core
    TR = NC * PCP + 1                 # table rows (+1 sentinel)
    SENT = TR - 1
    R1 = HID + 1       # bf16 table row: [h | ss]
    R2 = HID2 + 1
    NCH1, NCH2 = cfg["NCH1"], cfg["NCH2"]   # allgather chunks per layer
    CHR1 = PCP // NCH1                      # rows per chunk per core
    CHR2 = PCP // NCH2
    assert CHR1 * NCH1 == PCP and CHR2 * NCH2 == PCP

    x = np.asarray(x, np.float32)
    batch = np.asarray(batch, np.int64)
    src = np.asarray(edge_index[0], np.int64)
    dst = np.asarray(edge_index[1], np.int64)
    # self loops
    loop = np.arange(N, dtype=np.int64)
    src = np.concatenate([src, loop])
    dst = np.concatenate([dst, loop])

    counts = np.bincount(batch, minlength=G).astype(np.float64)

    # per-core degree-sorted permutations and global row ids. Table rows
    # are grouped (chunk, core, row-in-chunk) so each chunk's AllGather
    # writes a contiguous slice of the full table; layer 1 and 2 use
    # different chunk counts, hence separate row maps / index tables.
    row_of1 = np.empty(N, np.int64)      # global node -> layer-1 table row
    row_of2 = np.empty(N, np.int64)
    orders = []
    degs_sorted = np.zeros((NC, PCP), np.int64)
    for k in range(NC):
        lo, hi = k * PC, min((k + 1) * PC, N)
        nk = hi - lo
        mask = (dst >= lo) & (dst < hi)
        deg = np.bincount(dst[mask] - lo, minlength=nk)
        order = np.argsort(-deg, kind="stable")        # local rank -> local id
        inv = np.empty(nk, np.int64)
        inv[order] = np.arange(nk)
        row_of1[lo:hi] = (inv // CHR1) * (NC * CHR1) + k * CHR1 + (inv % CHR1)
        row_of2[lo:hi] = (inv // CHR2) * (NC * CHR2) + k * CHR2 + (inv % CHR2)
        orders.append(order)
        degs_sorted[k, :nk] = deg[order]

    # global per-block slot counts (identical program on every core)
    Ls = []
    for b in range(PB):
        Lb = int(degs_sorted[:, b * 128:(b + 1) * 128].max())
        Ls.append(max(Lb, 1))
    S = int(np.sum(Ls))
    offs = np.concatenate([[0], np.cumsum(Ls)]).astype(np.int64)

    import ml_dtypes
    BF = ml_dtypes.bfloat16

    # shared (replicated) weight uploads, attention vectors folded in
    KB = IND // 128
    W1e = np.concatenate([
        np.asarray(W1, np.float32),
        (np.asarray(W1, np.float32) @ np.asarray(a1_src, np.float32))[:, None],
        (np.asarray(W1, np.float32) @ np.asarray(a1_dst, np.float32))[:, None],
    ], axis=1)                                        # [IND, HID+2]
    W1u = np.ascontiguousarray(
        W1e.reshape(KB, 128, HID + 2)).astype(BF)
    W2e = np.concatenate([
        np.asarray(W2, np.float32),
        (np.asarray(W2, np.float32) @ np.asarray(a2_src, np.float32))[:, None],
        (np.asarray(W2, np.float32) @ np.asarray(a2_dst, np.float32))[:, None],
    ], axis=1).astype(BF)                             # [HID, HID2+2]
    b1r = np.tile(np.asarray(b1, np.float32)[None, :], (128, 1))
    b2r = np.tile(np.asarray(b2, np.float32)[None, :], (128, 1))
    WlBl = np.concatenate([np.asarray(Wl, np.float32),
                           np.asarray(bl, np.float32)[None, :]], axis=0)
    sent1 = np.zeros((1, HID + 2), np.uint8)    # [h_fp8 | ss_bf16] bytes
    sent1[0, HID:HID + 2] = np.array([SENT_SS], BF).view(np.uint8)
    sent2 = np.zeros((1, HID2 + 2), np.uint8)   # [h2_fp8 | ss2_bf16]
    sent2[0, HID2:HID2 + 2] = np.array([SENT_SS], BF).view(np.uint8)

    in_maps = []
    for k in range(NC):
        lo, hi = k * PC, min((k + 1) * PC, N)
        nk = hi - lo
        order = orders[k]

        # xT: [KB, 128, PCP] (feature-major columns in local-rank order)
        xs = np.zeros((PCP, IND), np.float32)
        xs[:nk] = x[lo:hi][order]
        xT = np.ascontiguousarray(xs.T.reshape(KB, 128, PCP)).astype(BF)

        # slot indices [128, S] -> table rows, sentinel padded (per layer)
        sidx1 = np.full((128, S), SENT, np.int64)
        sidx2 = np.full((128, S), SENT, np.int64)
        mask = (dst >= lo) & (dst < hi)
        es, ed = src[mask], dst[mask] - lo
        o = np.argsort(ed, kind="stable")
        es, ed = es[o], ed[o]
        deg = np.bincount(ed, minlength=nk)
        start = np.concatenate([[0], np.cumsum(deg)[:-1]])
        j = np.arange(len(ed)) - start[ed]            # slot within dest
        inv = np.empty(nk, np.int64)
        inv[order] = np.arange(nk)
        r = inv[ed]                                   # dest rank
        bb, pp = r // 128, r % 128
        col = offs[bb] + j
        sidx1[pp, col] = row_of1[es]
        sidx2[pp, col] = row_of2[es]

        # dma_gather index tables: per block b, idx i = l*128+p ->
        # sidx[p, l]; wrapped at [i%16, i//16] within the block's column
        # range, replicated across the 8 Q7 cores (16-partition groups)
        def wrap_idx(sidx):
            w = np.zeros((128, S * 8), np.int16)
            for b in range(PB):
                Lb = Ls[b]
                ilist = sidx[:, offs[b]:offs[b] + Lb].T.reshape(-1)
                blkw = ilist.reshape(-1, 16).T.astype(np.int16)
                w[:16, offs[b] * 8:(offs[b] + Lb) * 8] = blkw
            w[:] = np.tile(w[:16], (8, 1))
            return w

        idxw1 = wrap_idx(sidx1)
        idxw2 = wrap_idx(sidx2)

        # pooling one-hot with 1/count folded, zero rows for pad dests
        P = np.zeros((128, PB * G), np.float32)
        bg = batch[lo:hi][order]                      # graph id per rank
        rr = np.arange(nk)
        P[rr % 128, (rr // 128) * G + bg] = 1.0 / np.maximum(counts[bg], 1.0)

        in_maps.append(dict(
            xT=xT, W1u=W1u, W2u=W2e, b1r=b1r, b2r=b2r,
            Pp=P, sidx1=idxw1, sidx2=idxw2, sent1=sent1, sent2=sent2,
        ))

    meta = dict(PC=PC, PB=PB, PCP=PCP, TR=TR, R1=R1, R2=R2, KB=KB, S=S,
                Ls=Ls, offs=offs,
                WlBl=WlBl.astype(np.float32))
    return in_maps, meta


# ----------------------------------------------------------------------------
# Device program
# ----------------------------------------------------------------------------

def build_program(cfg, meta, debug_outs=False):
    N, IND, HID, HID2, OUT, G, NC = (cfg[k] for k in
                                     ("N", "IND", "HID", "HID2", "OUT",
                                      "G", "NCORES"))
    NCH1, NCH2, SSB = cfg["NCH1"], cfg["NCH2"], cfg["SSB"]
    PB, PCP, TR, R1, R2, KB, S = (meta[k] for k in
                                  ("PB", "PCP", "TR", "R1", "R2", "KB", "S"))
    Ls, offs = meta["Ls"], meta["offs"]
    CH1 = PB // NCH1     # blocks per allgather chunk (layer 1)
    CH2 = PB // NCH2
    CHR1 = CH1 * 128     # rows per chunk per core
    CHR2 = CH2 * 128
    assert CH1 * NCH1 == PB and CH2 * NCH2 == PB

    nc = bacc.Bacc("TRN2", target_bir_lowering=False, debug=False,
                   num_devices=NC)

    xT_d = nc.declare_dram_parameter("xT", [KB, 128, PCP], BF16, False)
    W1_d = nc.declare_dram_parameter("W1u", [KB, 128, HID + 2], BF16, False)
    W2_d = nc.declare_dram_parameter("W2u", [HID, HID2 + 2], BF16, False)
    b1r_d = nc.declare_dram_parameter("b1r", [128, HID], F32, False)
    b2r_d = nc.declare_dram_parameter("b2r", [128, HID2], F32, False)
    Pp_d = nc.declare_dram_parameter("Pp", [128, PB * G], F32, False)
    sidx1_d = nc.declare_dram_parameter("sidx1", [128, S * 8], I16, False)
    sidx2_d = nc.declare_dram_parameter("sidx2", [128, S * 8], I16, False)
    sent1_d = nc.declare_dram_parameter("sent1", [1, HID + 2], U8, False)
    sent2_d = nc.declare_dram_parameter("sent2", [1, HID2 + 2], U8, False)
    out_d = nc.declare_dram_parameter("out", [G, HID2], F32, True)
    if debug_outs:
        dbg_t1 = nc.declare_dram_parameter("dbg_t1", [TR, R1], BF16, True)
        dbg_sd1 = nc.declare_dram_parameter("dbg_sd1", [128, PB], F32, True)
        dbg_g = nc.declare_dram_parameter("dbg_g", [128, Ls[0] * R1], BF16,
                                          True)
        dbg_wl = nc.declare_dram_parameter("dbg_wl", [128, Ls[0]], F32, True)
        dbg_wex = nc.declare_dram_parameter("dbg_wex", [128, Ls[0]], F32,
                                            True)
        dbg_den = nc.declare_dram_parameter("dbg_den", [128, PB], F32, True)
        dbg_r1 = nc.declare_dram_parameter("dbg_r1", [PB * 128, HID], F32,
                                           True)

    shared = dict(addr_space="Shared") if NC > 4 else {}
    R1B = HID + 2        # layer-1 table row bytes: [h_fp8 | ss_bf16]
    T1sh = [nc.dram_tensor(f"T1shard{c}", [CHR1, R1B], U8)
            for c in range(NCH1)]
    T1full = nc.dram_tensor("T1full", [TR, R1B], U8, **shared)
    R2B = HID2 + 2       # layer-2 table row bytes: [h2_fp8 | ss2_bf16]
    T2sh = [nc.dram_tensor(f"T2shard{c}", [CHR2, R2B], U8)
            for c in range(NCH2)]
    T2full = nc.dram_tensor("T2full", [TR, R2B], U8, **shared)
    # 512B/256B-pitch padded copies of the tables for dma_gather (rows
    # must be a multiple of 256 bytes)
    T1pad = nc.dram_tensor("T1pad", [TR, 256], U8)
    T2pad = nc.dram_tensor("T2pad", [TR, 256], U8)

    groups = [list(range(NC))]

    with TileContext(nc) as tc:
        with (
            tc.tile_pool(name="const", bufs=1) as cp,
            tc.tile_pool(name="work", bufs=3) as wp,
            tc.tile_pool(name="gath", bufs=3) as gp,
            tc.tile_pool(name="wtp", bufs=2) as wtp,
            tc.tile_pool(name="psA", bufs=2, space="PSUM") as psA,
            tc.tile_pool(name="psB", bufs=2, space="PSUM") as psB,
            tc.tile_pool(name="psP", bufs=1, space="PSUM") as psP,
        ):
            # ---------------- constants to SBUF ----------------
            W1_sb = cp.tile([128, KB * (HID + 2)], BF16, tag="w1")
            W1v = W1_sb[:].rearrange("p (k h) -> p k h", h=HID + 2)
            nc.sync.dma_start(out=W1v, in_=W1_d[:].rearrange("k p h -> p k h"))
            W2_sb = cp.tile([HID, HID2 + 2], BF16, tag="w2")
            nc.sync.dma_start(out=W2_sb[:], in_=W2_d[:])
            b1r_sb = cp.tile([128, HID], F32, tag="b1r")
            nc.sync.dma_start(out=b1r_sb[:], in_=b1r_d[:])
            b2r_sb = cp.tile([128, HID2], F32, tag="b2r")
            nc.sync.dma_start(out=b2r_sb[:], in_=b2r_d[:])
            P_sb = cp.tile([128, PB * G], F32, tag="pp")
            nc.sync.dma_start(out=P_sb[:], in_=Pp_d[:])
            sidx1_sb = cp.tile([128, S * 8], I16, tag="sidx1")
            nc.sync.dma_start(out=sidx1_sb[:], in_=sidx1_d[:])
            sidx2_sb = cp.tile([128, S * 8], I16, tag="sidx2")
            nc.sync.dma_start(out=sidx2_sb[:], in_=sidx2_d[:])
            ident = cp.tile([128, 128], F32, tag="id")
            make_identity(nc, ident[:])

            T1sb = cp.tile([128, PB * R1B], U8, tag="t1")
            T2sb = cp.tile([128, PB * R2B], U8, tag="t2")
            sd1 = cp.tile([128, PB], F32, tag="sd1")
            sd2 = cp.tile([128, PB], F32, tag="sd2")

            nc.sync.dma_start(out=T1full[TR - 1:TR, :], in_=sent1_d[:])
            nc.sync.dma_start(out=T2full[TR - 1:TR, :], in_=sent2_d[:])
            nc.sync.dma_start(out=T1pad[TR - 1:TR, 0:R1B], in_=sent1_d[:])
            nc.sync.dma_start(out=T2pad[TR - 1:TR, 0:R2B], in_=sent2_d[:])

            # ---------------- phase A: h1/ss1/sd1, chunked allgather ------
            with tc.tile_pool(name="xpool", bufs=2) as xp:
                for c in range(PB):
                    xc = xp.tile([128, KB * 128], BF16, tag="xc")
                    xcv = xc[:].rearrange("p (k n) -> p k n", n=128)
                    nc.sync.dma_start(
                        out=xcv,
                        in_=xT_d[:, :, c * 128:(c + 1) * 128]
                        .rearrange("k p n -> p k n"))
                    ph = psA.tile([128, HID + 2], F32, tag="ph")
                    for kb in range(KB):
                        nc.tensor.matmul(
                            ph[:],
                            lhsT=xc[:, kb * 128:(kb + 1) * 128],
                            rhs=W1_sb[:, kb * (HID + 2):(kb + 1) * (HID + 2)],
                            start=(kb == 0), stop=(kb == KB - 1),
                        )
                    t1v = T1sb[:].rearrange("p (c w) -> p c w", w=R1B)
                    nc.vector.tensor_copy(
                        t1v[:, c, 0:HID].bitcast(FP8), ph[:, 0:HID])
                    nc.vector.tensor_copy(
                        t1v[:, c, HID:HID + 2].bitcast(BF16),
                        ph[:, HID:HID + 1])
                    nc.vector.tensor_copy(
                        sd1[:, c:c + 1], ph[:, HID + 1:HID + 2])
                    ci = c // CH1
                    nc.sync.dma_start(
                        out=T1sh[ci][(c % CH1) * 128:(c % CH1 + 1) * 128, :],
                        in_=T1sb[:, c * R1B:(c + 1) * R1B])
                    if c % CH1 == CH1 - 1:
                        nc.gpsimd.collective_compute(
                            "AllGather", ALU.bypass, replica_groups=groups,
                            ins=[T1sh[ci][:]],
                            outs=[T1full[ci * NC * CHR1:(ci + 1) * NC * CHR1,
                                         :]])
                        nc.sync.dma_start(
                            out=T1pad[ci * NC * CHR1:(ci + 1) * NC * CHR1,
                                      0:R1B],
                            in_=T1full[ci * NC * CHR1:(ci + 1) * NC * CHR1,
                                       :])

            # ---------------- phase B: GAT layer 1 ----------------
            # scores (gather + Prelu + Exp) are emitted one block ahead of
            # the per-edge weighting so DVE never stalls on ACT's queue
            def scores1(b):
                L = Ls[b]
                Gt = gp.tile([128, L * 256], U8, tag="g1")
                Gv = Gt[:].rearrange("p (l w) -> p l w", w=256)
                # dma_gather is capped at 1024 indices per instruction
                for s0 in range(0, L, 8):
                    Lc = min(8, L - s0)
                    nc.gpsimd.dma_gather(
                        Gv[:, s0:s0 + Lc, :], T1pad[:],
                        sidx1_sb[:, (offs[b] + s0) * 8:
                                 (offs[b] + s0 + Lc) * 8],
                        Lc * 128, Lc * 128, 256)
                ssv = Gv[:, :, HID:HID + 2].bitcast(BF16)
                wl_t = wp.tile([128, L], F32, tag="wl1")
                nc.scalar.activation(
                    wl_t[:], ssv[:, :, 0], AF.Prelu,
                    bias=sd1[:, b:b + 1], alpha=NEG_SLOPE)
                wex = wp.tile([128, L], F32, tag="we1")
                den = wp.tile([128, 1], F32, tag="den")
                nc.scalar.activation(
                    wex[:], wl_t[:], AF.Exp, accum_out=den[:])
                return Gv, wex, den

            sc = scores1(0)
            for b in range(PB):
                L = Ls[b]
                Gv, wex, den = sc
                if b + 1 < PB:
                    sc = scores1(b + 1)
                if debug_outs and b == 0:
                    nc.sync.dma_start(out=dbg_g[:], in_=Gt[:])
                    nc.sync.dma_start(out=dbg_wl[:], in_=wl_t[:])
                    nc.sync.dma_start(out=dbg_wex[:], in_=wex[:])
                if debug_outs:
                    nc.sync.dma_start(out=dbg_den[:, b:b + 1], in_=den[:])
                # per-edge weighting: alternate blocks between DVE and
                # gpsimd (Pool) to balance the two engines
                wt = wtp.tile([128, L * HID], BF16, tag="wt")
                wtv = wt[:].rearrange("p (l f) -> p l f", f=HID)
                hv = Gv[:, :, 0:HID].bitcast(FP8)
                # split the per-edge weighting so ACT (per-slot scale-copy,
                # ~2.2x slower per slot) and DVE (one op for the rest +
                # the reduce) finish together
                a = max(0, min(L, int(0.63 * L) - 1))
                for l in range(a):
                    nc.scalar.activation(
                        wtv[:, l, :], hv[:, l, :], AF.Copy,
                        scale=wex[:, l:l + 1])
                if a < L:
                    nc.vector.tensor_tensor(
                        out=wtv[:, a:L, :], in0=hv[:, a:L, :],
                        in1=wex[:, a:L, None].to_broadcast(
                            [128, L - a, HID]),
                        op=ALU.mult)
                o_t = wp.tile([128, HID], F32, tag="o1")
                nc.vector.tensor_reduce(
                    out=o_t[:],
                    in_=wt[:].rearrange("p (l f) -> p f l", f=HID),
                    axis=mybir.AxisListType.X, op=ALU.add)
                rec = wp.tile([128, 1], F32, tag="rec")
                nc.vector.reciprocal(rec[:], den[:])
                ob = wp.tile([128, HID], F32, tag="ob")
                nc.vector.scalar_tensor_tensor(
                    out=ob[:], in0=o_t[:], scalar=rec[:], in1=b1r_sb[:],
                    op0=ALU.mult, op1=ALU.add)
                r1 = wp.tile([128, HID], F32, tag="r1")
                nc.scalar.activation(r1[:], ob[:], AF.Relu)
                if debug_outs:
                    nc.sync.dma_start(
                        out=dbg_r1[b * 128:(b + 1) * 128, :], in_=r1[:])
                # transpose -> [h2 | ss2 | sd2] = r1 @ W2e
                pT = psB.tile([128, HID], F32, tag="tr")
                nc.tensor.transpose(pT[:], r1[:], identity=ident[:])
                r1T = wp.tile([128, HID], BF16, tag="r1T")
                nc.vector.tensor_copy(r1T[:], pT[:])
                ph2 = psB.tile([128, HID2 + 2], F32, tag="tr2")
                nc.tensor.matmul(ph2[:], lhsT=r1T[:], rhs=W2_sb[:],
                                 start=True, stop=True)
                t2v = T2sb[:].rearrange("p (c w) -> p c w", w=R2B)
                nc.vector.tensor_copy(
                    t2v[:, b, 0:HID2].bitcast(FP8), ph2[:, 0:HID2])
                nc.vector.tensor_copy(
                    t2v[:, b, HID2:HID2 + 2].bitcast(BF16),
                    ph2[:, HID2:HID2 + 1])
                nc.vector.tensor_copy(
                    sd2[:, b:b + 1], ph2[:, HID2 + 1:HID2 + 2])
                ci = b // CH2
                nc.sync.dma_start(
                    out=T2sh[ci][(b % CH2) * 128:(b % CH2 + 1) * 128, :],
                    in_=T2sb[:, b * R2B:(b + 1) * R2B])
                if b % CH2 == CH2 - 1:
                    nc.gpsimd.collective_compute(
                        "AllGather", ALU.bypass, replica_groups=groups,
                        ins=[T2sh[ci][:]],
                        outs=[T2full[ci * NC * CHR2:(ci + 1) * NC * CHR2,
                                     :]])
                    nc.sync.dma_start(
                        out=T2pad[ci * NC * CHR2:(ci + 1) * NC * CHR2,
                                  0:R2B],
                        in_=T2full[ci * NC * CHR2:(ci + 1) * NC * CHR2, :])

            # ---------------- phase C: GAT layer 2 + pooling ----------------
            pool_ps = psP.tile([G, HID2], F32, tag="pool")

            def scores2(b):
                L = Ls[b]
                Gt = gp.tile([128, L * 256], U8, tag="g2")
                Gv = Gt[:].rearrange("p (l w) -> p l w", w=256)
                for s0 in range(0, L, 8):
                    Lc = min(8, L - s0)
                    nc.gpsimd.dma_gather(
                        Gv[:, s0:s0 + Lc, :], T2pad[:],
                        sidx2_sb[:, (offs[b] + s0) * 8:
                                 (offs[b] + s0 + Lc) * 8],
                        Lc * 128, Lc * 128, 256)
                ssv = Gv[:, :, HID2:HID2 + 2].bitcast(BF16)
                wl_t = wp.tile([128, L], F32, tag="wl1")
                nc.scalar.activation(
                    wl_t[:], ssv[:, :, 0], AF.Prelu,
                    bias=sd2[:, b:b + 1], alpha=NEG_SLOPE)
                wex = wp.tile([128, L], F32, tag="we1")
                den = wp.tile([128, 1], F32, tag="den")
                nc.scalar.activation(
                    wex[:], wl_t[:], AF.Exp, accum_out=den[:])
                return Gv, wex, den

            sc = scores2(0)
            for b in range(PB):
                L = Ls[b]
                Gv, wex, den = sc
                if b + 1 < PB:
                    sc = scores2(b + 1)
                wt = wtp.tile([128, L * HID2], BF16, tag="wt2")
                nc.vector.tensor_tensor(
                    out=wt[:], in0=Gv[:, :, 0:HID2].bitcast(FP8),
                    in1=wex[:, :, None].to_broadcast([128, L, HID2]),
                    op=ALU.mult)
                o_t = wp.tile([128, HID2], F32, tag="o2")
                nc.vector.tensor_reduce(
                    out=o_t[:],
                    in_=wt[:].rearrange("p (l f) -> p f l", f=HID2),
                    axis=mybir.AxisListType.X, op=ALU.add)
                rec = wp.tile([128, 1], F32, tag="rec")
                nc.vector.reciprocal(rec[:], den[:])
                ob = wp.tile([128, HID2], F32, tag="ob2")
                nc.vector.scalar_tensor_tensor(
                    out=ob[:], in0=o_t[:], scalar=rec[:], in1=b2r_sb[:],
                    op0=ALU.mult, op1=ALU.add)
                r2 = wp.tile([128, HID2], F32, tag="r2")
                nc.scalar.activation(r2[:], ob[:], AF.Relu)
                nc.tensor.matmul(
                    pool_ps[:], lhsT=P_sb[:, b * G:(b + 1) * G], rhs=r2[:],
                    start=(b == 0), stop=(b == PB - 1))

            # each core returns its pooled partial sums; the host adds the
            # 8 partials and applies the tiny [64,2] head
            pooled = wp.tile([G, HID2], F32, tag="pool")
            nc.vector.tensor_copy(pooled[:], pool_ps[:])
            nc.sync.dma_start(out=out_d[:], in_=pooled[:])
            if debug_outs:
                nc.sync.dma_start(out=dbg_t1[:], in_=T1full[:])
                nc.sync.dma_start(out=dbg_sd1[:], in_=sd1[:])

    nc.compile()
    return nc


# ----------------------------------------------------------------------------
# Entry point
# ----------------------------------------------------------------------------

LAST_RESULTS = None
LAST_BUILD = None


def kernel(**inputs):
    global LAST_RESULTS, LAST_BUILD
    cfg = full_cfg()
    in_maps, meta = preprocess(cfg=cfg, **inputs)
    nc = build_program(cfg, meta)
    LAST_BUILD = (nc, in_maps, meta)
    res = run_bass_kernel_spmd(nc, in_maps, core_ids=list(range(cfg["NCORES"])))
    LAST_RESULTS = res
    pooled = np.sum([np.asarray(r["out"], np.float32) for r in res.results],
                    axis=0)
    WlBl = meta["WlBl"]
    return (pooled @ WlBl[:-1] + WlBl[-1]).astype(np.float32)


# revision 56
# speedup vs baseline: 1.0577x; 1.0158x over previous
"""Trainium2 Bass kernel for a 2-layer GAT + global mean pool + linear head.

Math (matches PyG GATConv, eval mode, single head, add_self_loops=True):
  h   = x @ W
  e_k = lrelu(ss[src_k] + sd[dst_k]),  ss = h@a_src, sd = h@a_dst
  alpha = softmax over incoming edges of each dst (self-loop included)
  out[d] = sum_k alpha_k h[src_k] + b
Two GAT layers (512->128, 128->64) with ReLU, then per-graph mean pool
over `batch` and a final [64,2] linear.

Strategy (8 NeuronCores, full inputs in / full output out):
  * Destination nodes sharded across cores (2500/core), sources arbitrary.
  * a_src/a_dst are folded into the weight matrices host-side
    (W1e = [W1 | W1@a1_src | W1@a1_dst]), so the phase-A matmul (bf16 on
    PE) directly produces [h | ss | sd] per node. Table rows are packed
    as bytes [h_fp8e4m3 | ss_bf16] (130B layer 1, 66B layer 2),
    AllGathered compactly (layer 2 in chunks overlapped with phase-B
    compute), then locally expanded to a 256B-pitch copy because
    dma_gather requires 256B-multiple rows.
  * Edges grouped per destination into fixed "slots" (padded with a
    sentinel table row whose score ~ -60 contributes ~0 to the softmax),
    destinations per-partition, degree-sorted so each 128-dest block has
    a small uniform slot count L. Source rows are fetched with
    gpsimd.dma_gather (true multi-index gather; ucode caps one
    instruction at 1024 indices, so each block issues ceil(L*128/1024)
    sub-gathers into slices of one tile). NOTE: indirect_dma_start with
    a multi-column offset AP does NOT do a per-element gather on real
    HW (it fetches consecutive rows from each partition's first index).
  * Attention on ACT: Prelu (bias = per-partition sd, alpha=0.2;
    Prelu shares the "exp_and_others" act table with Exp/Relu so no
    table thrashing) then Exp with accum_out = softmax denominator.
    Per-edge weighting is split ~63/37 between ACT (per-slot scale-copy)
    and DVE (one broadcast multiply) so both engines finish together;
    the segment sum is a strided tensor_reduce on DVE.
  * Per-graph pooling one-hots (with 1/count folded in) are host-built
    and applied on PE; each core returns its pooled partials and the
    host sums them and applies the [64,2] head (cheaper than an
    on-device AllReduce + replicated linear tail).
"""

import math
import numpy as np

import concourse.bass as bass
import concourse.bacc as bacc
import concourse.mybir as mybir
from concourse.tile import TileContext
from concourse.masks import make_identity
from concourse.bass_utils import run_bass_kernel_spmd

F32 = mybir.dt.float32
BF16 = mybir.dt.bfloat16
I32 = mybir.dt.int32
I16 = mybir.dt.int16
U8 = mybir.dt.uint8
FP8 = mybir.dt.float8e4
AF = mybir.ActivationFunctionType
ALU = mybir.AluOpType

NEG_SLOPE = 0.2
SENT_SS = -60.0  # sentinel row score: exp(lrelu(-60+sd)) ~ e^-12 -> harmless


def full_cfg():
    return dict(N=20000, IND=512, HID=128, HID2=64, OUT=2, G=16, NCORES=8,
                NCH1=2, NCH2=4, SSB=5)


# ----------------------------------------------------------------------------
# Host-side preprocessing
# ----------------------------------------------------------------------------

def preprocess(x, edge_index, batch, W1, a1_src, a1_dst, b1,
               W2, a2_src, a2_dst, b2, Wl, bl, cfg):
    N, IND, HID, HID2, OUT, G, NC = (cfg[k] for k in
                                     ("N", "IND", "HID", "HID2", "OUT", "G",
                                      "NCORES"))
    PC = math.ceil(N / NC)            # real dests per core
    PB = math.ceil(PC / 128)          # dest blocks per core
    PCP = PB * 128                    # padded dests per core
    TR = NC * PCP + 1                 # table rows (+1 sentinel)
    SENT = TR - 1
    R1 = HID + 1       # bf16 table row: [h | ss]
    R2 = HID2 + 1
    NCH1, NCH2 = cfg["NCH1"], cfg["NCH2"]   # allgather chunks per layer
    CHR1 = PCP // NCH1                      # rows per chunk per core
    CHR2 = PCP // NCH2
    assert CHR1 * NCH1 == PCP and CHR2 * NCH2 == PCP

    x = np.asarray(x, np.float32)
    batch = np.asarray(batch, np.int64)
    src = np.asarray(edge_index[0], np.int64)
    dst = np.asarray(edge_index[1], np.int64)
    # self loops
    loop = np.arange(N, dtype=np.int64)
    src = np.concatenate([src, loop])
    dst = np.concatenate([dst, loop])

    counts = np.bincount(batch, minlength=G).astype(np.float64)

    # per-core degree-sorted permutations and global row ids. Table rows
    # are grouped (chunk, core, row-in-chunk) so each chunk's AllGather
    # writes a contiguous slice of the full table; layer 1 and 2 use
    # different chunk counts, hence separate row maps / index tables.
    row_of1 = np.empty(N, np.int64)      # global node -> layer-1 table row
    row_of2 = np.empty(N, np.int64)
    orders = []
    degs_sorted = np.zeros((NC, PCP), np.int64)
    for k in range(NC):
        lo, hi = k * PC, min((k + 1) * PC, N)
        nk = hi - lo
        mask = (dst >= lo) & (dst < hi)
        deg = np.bincount(dst[mask] - lo, minlength=nk)
        order = np.argsort(-deg, kind="stable")        # local rank -> local id
        inv = np.empty(nk, np.int64)
        inv[order] = np.arange(nk)
        row_of1[lo:hi] = (inv // CHR1) * (NC * CHR1) + k * CHR1 + (inv % CHR1)
        row_of2[lo:hi] = (inv // CHR2) * (NC * CHR2) + k * CHR2 + (inv % CHR2)
        orders.append(order)
        degs_sorted[k, :nk] = deg[order]

    # global per-block slot counts (identical program on every core)
    Ls = []
    for b in range(PB):
        Lb = int(degs_sorted[:, b * 128:(b + 1) * 128].max())
        Ls.append(max(Lb, 1))
    S = int(np.sum(Ls))
    offs = np.concatenate([[0], np.cumsum(Ls)]).astype(np.int64)

    import ml_dtypes
    BF = ml_dtypes.bfloat16

    # shared (replicated) weight uploads, attention vectors folded in
    KB = IND // 128
    W1e = np.concatenate([
        np.asarray(W1, np.float32),
        (np.asarray(W1, np.float32) @ np.asarray(a1_src, np.float32))[:, None],
        (np.asarray(W1, np.float32) @ np.asarray(a1_dst, np.float32))[:, None],
    ], axis=1)                                        # [IND, HID+2]
    W1u = np.ascontiguousarray(
        W1e.reshape(KB, 128, HID + 2)).astype(BF)
    W2e = np.concatenate([
        np.asarray(W2, np.float32),
        (np.asarray(W2, np.float32) @ np.asarray(a2_src, np.float32))[:, None],
        (np.asarray(W2, np.float32) @ np.asarray(a2_dst, np.float32))[:, None],
    ], axis=1).astype(BF)                             # [HID, HID2+2]
    b1r = np.tile(np.asarray(b1, np.float32)[None, :], (128, 1))
    b2r = np.tile(np.asarray(b2, np.float32)[None, :], (128, 1))
    WlBl = np.concatenate([np.asarray(Wl, np.float32),
                           np.asarray(bl, np.float32)[None, :]], axis=0)
    sent1 = np.zeros((1, HID + 2), np.uint8)    # [h_fp8 | ss_bf16] bytes
    sent1[0, HID:HID + 2] = np.array([SENT_SS], BF).view(np.uint8)
    sent2 = np.zeros((1, HID2 + 2), np.uint8)   # [h2_fp8 | ss2_bf16]
    sent2[0, HID2:HID2 + 2] = np.array([SENT_SS], BF).view(np.uint8)

    in_maps = []
    for k in range(NC):
        lo, hi = k * PC, min((k + 1) * PC, N)
        nk = hi - lo
        order = orders[k]

        # xT: [KB, 128, PCP] (feature-major columns in local-rank order)
        xs = np.zeros((PCP, IND), np.float32)
        xs[:nk] = x[lo:hi][order]
        xT = np.ascontiguousarray(xs.T.reshape(KB, 128, PCP)).astype(BF)

        # slot indices [128, S] -> table rows, sentinel padded (per layer)
        sidx1 = np.full((128, S), SENT, np.int64)
        sidx2 = np.full((128, S), SENT, np.int64)
        mask = (dst >= lo) & (dst < hi)
        es, ed = src[mask], dst[mask] - lo
        o = np.argsort(ed, kind="stable")
        es, ed = es[o], ed[o]
        deg = np.bincount(ed, minlength=nk)
        start = np.concatenate([[0], np.cumsum(deg)[:-1]])
        j = np.arange(len(ed)) - start[ed]            # slot within dest
        inv = np.empty(nk, np.int64)
        inv[order] = np.arange(nk)
        r = inv[ed]                                   # dest rank
        bb, pp = r // 128, r % 128
        col = offs[bb] + j
        sidx1[pp, col] = row_of1[es]
        sidx2[pp, col] = row_of2[es]

        # dma_gather index tables: per block b, idx i = l*128+p ->
        # sidx[p, l]; wrapped at [i%16, i//16] within the block's column
        # range, replicated across the 8 Q7 cores (16-partition groups)
        def wrap_idx(sidx):
            w = np.zeros((128, S * 8), np.int16)
            for b in range(PB):
                Lb = Ls[b]
                ilist = sidx[:, offs[b]:offs[b] + Lb].T.reshape(-1)
                blkw = ilist.reshape(-1, 16).T.astype(np.int16)
                w[:16, offs[b] * 8:(offs[b] + Lb) * 8] = blkw
            w[:] = np.tile(w[:16], (8, 1))
            return w

        idxw1 = wrap_idx(sidx1)
        idxw2 = wrap_idx(sidx2)

        # pooling one-hot with 1/count folded, zero rows for pad dests
        P = np.zeros((128, PB * G), np.float32)
        bg = batch[lo:hi][order]                      # graph id per rank
        rr = np.arange(nk)
        P[rr % 128, (rr // 128) * G + bg] = 1.0 / np.maximum(counts[bg], 1.0)

        in_maps.append(dict(
            xT=xT, W1u=W1u, W2u=W2e, b1r=b1r, b2r=b2r,
            Pp=P, sidx1=idxw1, sidx2=idxw2, sent1=sent1, sent2=sent2,
        ))

    meta = dict(PC=PC, PB=PB, PCP=PCP, TR=TR, R1=R1, R2=R2, KB=KB, S=S,
                Ls=Ls, offs=offs,
                WlBl=WlBl.astype(np.float32))
    return in_maps, meta


# ----------------------------------------------------------------------------
# Device program
# ----------------------------------------------------------------------------

def build_program(cfg, meta, debug_outs=False):
    N, IND, HID, HID2, OUT, G, NC = (cfg[k] for k in
                                     ("N", "IND", "HID", "HID2", "OUT",
                                      "G", "NCORES"))
    NCH1, NCH2, SSB = cfg["NCH1"], cfg["NCH2"], cfg["SSB"]
    PB, PCP, TR, R1, R2, KB, S = (meta[k] for k in
                                  ("PB", "PCP", "TR", "R1", "R2", "KB", "S"))
    Ls, offs = meta["Ls"], meta["offs"]
    CH1 = PB // NCH1     # blocks per allgather chunk (layer 1)
    CH2 = PB // NCH2
    CHR1 = CH1 * 128     # rows per chunk per core
    CHR2 = CH2 * 128
    assert CH1 * NCH1 == PB and CH2 * NCH2 == PB

    nc = bacc.Bacc("TRN2", target_bir_lowering=False, debug=False,
                   num_devices=NC)

    xT_d = nc.declare_dram_parameter("xT", [KB, 128, PCP], BF16, False)
    W1_d = nc.declare_dram_parameter("W1u", [KB, 128, HID + 2], BF16, False)
    W2_d = nc.declare_dram_parameter("W2u", [HID, HID2 + 2], BF16, False)
    b1r_d = nc.declare_dram_parameter("b1r", [128, HID], F32, False)
    b2r_d = nc.declare_dram_parameter("b2r", [128, HID2], F32, False)
    Pp_d = nc.declare_dram_parameter("Pp", [128, PB * G], F32, False)
    sidx1_d = nc.declare_dram_parameter("sidx1", [128, S * 8], I16, False)
    sidx2_d = nc.declare_dram_parameter("sidx2", [128, S * 8], I16, False)
    sent1_d = nc.declare_dram_parameter("sent1", [1, HID + 2], U8, False)
    sent2_d = nc.declare_dram_parameter("sent2", [1, HID2 + 2], U8, False)
    out_d = nc.declare_dram_parameter("out", [G, HID2], F32, True)
    if debug_outs:
        dbg_t1 = nc.declare_dram_parameter("dbg_t1", [TR, R1], BF16, True)
        dbg_sd1 = nc.declare_dram_parameter("dbg_sd1", [128, PB], F32, True)
        dbg_g = nc.declare_dram_parameter("dbg_g", [128, Ls[0] * R1], BF16,
                                          True)
        dbg_wl = nc.declare_dram_parameter("dbg_wl", [128, Ls[0]], F32, True)
        dbg_wex = nc.declare_dram_parameter("dbg_wex", [128, Ls[0]], F32,
                                            True)
        dbg_den = nc.declare_dram_parameter("dbg_den", [128, PB], F32, True)
        dbg_r1 = nc.declare_dram_parameter("dbg_r1", [PB * 128, HID], F32,
                                           True)

    shared = dict(addr_space="Shared") if NC > 4 else {}
    R1B = HID + 2        # layer-1 table row bytes: [h_fp8 | ss_bf16]
    T1sh = [nc.dram_tensor(f"T1shard{c}", [CHR1, R1B], U8)
            for c in range(NCH1)]
    T1full = nc.dram_tensor("T1full", [TR, R1B], U8, **shared)
    R2B = HID2 + 2       # layer-2 table row bytes: [h2_fp8 | ss2_bf16]
    T2sh = [nc.dram_tensor(f"T2shard{c}", [CHR2, R2B], U8)
            for c in range(NCH2)]
    T2full = nc.dram_tensor("T2full", [TR, R2B], U8, **shared)
    # 512B/256B-pitch padded copies of the tables for dma_gather (rows
    # must be a multiple of 256 bytes)
    T1pad = nc.dram_tensor("T1pad", [TR, 256], U8)
    T2pad = nc.dram_tensor("T2pad", [TR, 256], U8)

    groups = [list(range(NC))]

    with TileContext(nc) as tc:
        with (
            tc.tile_pool(name="const", bufs=1) as cp,
            tc.tile_pool(name="work", bufs=3) as wp,
            tc.tile_pool(name="gath", bufs=3) as gp,
            tc.tile_pool(name="wtp", bufs=2) as wtp,
            tc.tile_pool(name="psA", bufs=2, space="PSUM") as psA,
            tc.tile_pool(name="psB", bufs=2, space="PSUM") as psB,
            tc.tile_pool(name="psP", bufs=1, space="PSUM") as psP,
        ):
            # ---------------- constants to SBUF ----------------
            W1_sb = cp.tile([128, KB * (HID + 2)], BF16, tag="w1")
            W1v = W1_sb[:].rearrange("p (k h) -> p k h", h=HID + 2)
            nc.sync.dma_start(out=W1v, in_=W1_d[:].rearrange("k p h -> p k h"))
            W2_sb = cp.tile([HID, HID2 + 2], BF16, tag="w2")
            nc.sync.dma_start(out=W2_sb[:], in_=W2_d[:])
            b1r_sb = cp.tile([128, HID], F32, tag="b1r")
            nc.sync.dma_start(out=b1r_sb[:], in_=b1r_d[:])
            b2r_sb = cp.tile([128, HID2], F32, tag="b2r")
            nc.sync.dma_start(out=b2r_sb[:], in_=b2r_d[:])
            P_sb = cp.tile([128, PB * G], F32, tag="pp")
            nc.sync.dma_start(out=P_sb[:], in_=Pp_d[:])
            sidx1_sb = cp.tile([128, S * 8], I16, tag="sidx1")
            nc.sync.dma_start(out=sidx1_sb[:], in_=sidx1_d[:])
            sidx2_sb = cp.tile([128, S * 8], I16, tag="sidx2")
            nc.sync.dma_start(out=sidx2_sb[:], in_=sidx2_d[:])
            ident = cp.tile([128, 128], F32, tag="id")
            make_identity(nc, ident[:])

            T1sb = cp.tile([128, PB * R1B], U8, tag="t1")
            T2sb = cp.tile([128, PB * R2B], U8, tag="t2")
            sd1 = cp.tile([128, PB], F32, tag="sd1")
            sd2 = cp.tile([128, PB], F32, tag="sd2")

            nc.sync.dma_start(out=T1full[TR - 1:TR, :], in_=sent1_d[:])
            nc.sync.dma_start(out=T2full[TR - 1:TR, :], in_=sent2_d[:])
            nc.sync.dma_start(out=T1pad[TR - 1:TR, 0:R1B], in_=sent1_d[:])
            nc.sync.dma_start(out=T2pad[TR - 1:TR, 0:R2B], in_=sent2_d[:])

            # ---------------- phase A: h1/ss1/sd1, chunked allgather ------
            with tc.tile_pool(name="xpool", bufs=2) as xp:
                for c in range(PB):
                    xc = xp.tile([128, KB * 128], BF16, tag="xc")
                    xcv = xc[:].rearrange("p (k n) -> p k n", n=128)
                    nc.sync.dma_start(
                        out=xcv,
                        in_=xT_d[:, :, c * 128:(c + 1) * 128]
                        .rearrange("k p n -> p k n"))
                    ph = psA.tile([128, HID + 2], F32, tag="ph")
                    for kb in range(KB):
                        nc.tensor.matmul(
                            ph[:],
                            lhsT=xc[:, kb * 128:(kb + 1) * 128],
                            rhs=W1_sb[:, kb * (HID + 2):(kb + 1) * (HID + 2)],
                            start=(kb == 0), stop=(kb == KB - 1),
                        )
                    t1v = T1sb[:].rearrange("p (c w) -> p c w", w=R1B)
                    nc.vector.tensor_copy(
                        t1v[:, c, 0:HID].bitcast(FP8), ph[:, 0:HID])
                    nc.vector.tensor_copy(
                        t1v[:, c, HID:HID + 2].bitcast(BF16),
                        ph[:, HID:HID + 1])
                    nc.vector.tensor_copy(
                        sd1[:, c:c + 1], ph[:, HID + 1:HID + 2])
                    ci = c // CH1
                    nc.sync.dma_start(
                        out=T1sh[ci][(c % CH1) * 128:(c % CH1 + 1) * 128, :],
                        in_=T1sb[:, c * R1B:(c + 1) * R1B])
                    if c % CH1 == CH1 - 1:
                        nc.gpsimd.collective_compute(
                            "AllGather", ALU.bypass, replica_groups=groups,
                            ins=[T1sh[ci][:]],
                            outs=[T1full[ci * NC * CHR1:(ci + 1) * NC * CHR1,
                                         :]])
                        nc.sync.dma_start(
                            out=T1pad[ci * NC * CHR1:(ci + 1) * NC * CHR1,
                                      0:R1B],
                            in_=T1full[ci * NC * CHR1:(ci + 1) * NC * CHR1,
                                       :])

            # ---------------- phase B: GAT layer 1 ----------------
            # scores (gather + Prelu + Exp) are emitted one block ahead of
            # the per-edge weighting so DVE never stalls on ACT's queue
            def scores1(b):
                L = Ls[b]
                Gt = gp.tile([128, L * 256], U8, tag="g1")
                Gv = Gt[:].rearrange("p (l w) -> p l w", w=256)
                # dma_gather is capped at 1024 indices per instruction
                for s0 in range(0, L, 8):
                    Lc = min(8, L - s0)
                    nc.gpsimd.dma_gather(
                        Gv[:, s0:s0 + Lc, :], T1pad[:],
                        sidx1_sb[:, (offs[b] + s0) * 8:
                                 (offs[b] + s0 + Lc) * 8],
                        Lc * 128, Lc * 128, 256)
                ssv = Gv[:, :, HID:HID + 2].bitcast(BF16)
                wl_t = wp.tile([128, L], F32, tag="wl1")
                nc.scalar.activation(
                    wl_t[:], ssv[:, :, 0], AF.Prelu,
                    bias=sd1[:, b:b + 1], alpha=NEG_SLOPE)
                wex = wp.tile([128, L], F32, tag="we1")
                den = wp.tile([128, 1], F32, tag="den")
                nc.scalar.activation(
                    wex[:], wl_t[:], AF.Exp, accum_out=den[:])
                return Gv, wex, den

            sc = scores1(0)
            for b in range(PB):
                L = Ls[b]
                Gv, wex, den = sc
                if b + 1 < PB:
                    sc = scores1(b + 1)
                if debug_outs and b == 0:
                    nc.sync.dma_start(out=dbg_g[:], in_=Gt[:])
                    nc.sync.dma_start(out=dbg_wl[:], in_=wl_t[:])
                    nc.sync.dma_start(out=dbg_wex[:], in_=wex[:])
                if debug_outs:
                    nc.sync.dma_start(out=dbg_den[:, b:b + 1], in_=den[:])
                # per-edge weighting: alternate blocks between DVE and
                # gpsimd (Pool) to balance the two engines
                wt = wtp.tile([128, L * HID], BF16, tag="wt")
                wtv = wt[:].rearrange("p (l f) -> p l f", f=HID)
                hv = Gv[:, :, 0:HID].bitcast(FP8)
                # split the per-edge weighting so ACT (per-slot scale-copy,
                # ~2.2x slower per slot) and DVE (one op for the rest +
                # the reduce) finish together
                a = max(0, min(L, int(0.63 * L) - 1))
                for l in range(a):
                    nc.scalar.activation(
                        wtv[:, l, :], hv[:, l, :], AF.Copy,
                        scale=wex[:, l:l + 1])
                if a < L:
                    nc.vector.tensor_tensor(
                        out=wtv[:, a:L, :], in0=hv[:, a:L, :],
                        in1=wex[:, a:L, None].to_broadcast(
                            [128, L - a, HID]),
                        op=ALU.mult)
                o_t = wp.tile([128, HID], F32, tag="o1")
                nc.vector.tensor_reduce(
                    out=o_t[:],
                    in_=wt[:].rearrange("p (l f) -> p f l", f=HID),
                    axis=mybir.AxisListType.X, op=ALU.add)
                rec = wp.tile([128, 1], F32, tag="rec")
                nc.vector.reciprocal(rec[:], den[:])
                ob = wp.tile([128, HID], F32, tag="ob")
                nc.vector.scalar_tensor_tensor(
                    out=ob[:], in0=o_t[:], scalar=rec[:], in1=b1r_sb[:],
                    op0=ALU.mult, op1=ALU.add)
                r1 = wp.tile([128, HID], F32, tag="r1")
                nc.scalar.activation(r1[:], ob[:], AF.Relu)
                if debug_outs:
                    nc.sync.dma_start(
                        out=dbg_r1[b * 128:(b + 1) * 128, :], in_=r1[:])
                # transpose -> [h2 | ss2 | sd2] = r1 @ W2e
                pT = psB.tile([128, HID], F32, tag="tr")
                nc.tensor.transpose(pT[:], r1[:], identity=ident[:])
                r1T = wp.tile([128, HID], BF16, tag="r1T")
                nc.vector.tensor_copy(r1T[:], pT[:])
                ph2 = psB.tile([128, HID2 + 2], F32, tag="tr2")
                nc.tensor.matmul(ph2[:], lhsT=r1T[:], rhs=W2_sb[:],
                                 start=True, stop=True)
                t2v = T2sb[:].rearrange("p (c w) -> p c w", w=R2B)
                nc.vector.tensor_copy(
                    t2v[:, b, 0:HID2].bitcast(FP8), ph2[:, 0:HID2])
                nc.vector.tensor_copy(
                    t2v[:, b, HID2:HID2 + 2].bitcast(BF16),
                    ph2[:, HID2:HID2 + 1])
                nc.vector.tensor_copy(
                    sd2[:, b:b + 1], ph2[:, HID2 + 1:HID2 + 2])
                ci = b // CH2
                nc.sync.dma_start(
                    out=T2sh[ci][(b % CH2) * 128:(b % CH2 + 1) * 128, :],
                    in_=T2sb[:, b * R2B:(b + 1) * R2B])
                if b % CH2 == CH2 - 1:
                    nc.gpsimd.collective_compute(
                        "AllGather", ALU.bypass, replica_groups=groups,
                        ins=[T2sh[ci][:]],
                        outs=[T2full[ci * NC * CHR2:(ci + 1) * NC * CHR2,
                                     :]])
                    nc.sync.dma_start(
                        out=T2pad[ci * NC * CHR2:(ci + 1) * NC * CHR2,
                                  0:R2B],
                        in_=T2full[ci * NC * CHR2:(ci + 1) * NC * CHR2, :])

            # ---------------- phase C: GAT layer 2 + pooling ----------------
            pool_ps = psP.tile([G, HID2], F32, tag="pool")

            def scores2(b):
                L = Ls[b]
                Gt = gp.tile([128, L * 256], U8, tag="g2")
                Gv = Gt[:].rearrange("p (l w) -> p l w", w=256)
                for s0 in range(0, L, 8):
                    Lc = min(8, L - s0)
                    nc.gpsimd.dma_gather(
                        Gv[:, s0:s0 + Lc, :], T2pad[:],
                        sidx2_sb[:, (offs[b] + s0) * 8:
                                 (offs[b] + s0 + Lc) * 8],
                        Lc * 128, Lc * 128, 256)
                ssv = Gv[:, :, HID2:HID2 + 2].bitcast(BF16)
                wl_t = wp.tile([128, L], F32, tag="wl1")
                nc.scalar.activation(
                    wl_t[:], ssv[:, :, 0], AF.Prelu,
                    bias=sd2[:, b:b + 1], alpha=NEG_SLOPE)
                wex = wp.tile([128, L], F32, tag="we1")
                den = wp.tile([128, 1], F32, tag="den")
                nc.scalar.activation(
                    wex[:], wl_t[:], AF.Exp, accum_out=den[:])
                return Gv, wex, den

            sc = scores2(0)
            for b in range(PB):
                L = Ls[b]
                Gv, wex, den = sc
                if b + 1 < PB:
                    sc = scores2(b + 1)
                wt = wtp.tile([128, L * HID2], BF16, tag="wt2")
                nc.vector.tensor_tensor(
                    out=wt[:], in0=Gv[:, :, 0:HID2].bitcast(FP8),
                    in1=wex[:, :, None].to_broadcast([128, L, HID2]),
                    op=ALU.mult)
                o_t = wp.tile([128, HID2], F32, tag="o2")
                nc.vector.tensor_reduce(
                    out=o_t[:],
                    in_=wt[:].rearrange("p (l f) -> p f l", f=HID2),
                    axis=mybir.AxisListType.X, op=ALU.add)
                rec = wp.tile([128, 1], F32, tag="rec")
                nc.vector.reciprocal(rec[:], den[:])
                ob = wp.tile([128, HID2], F32, tag="ob2")
                nc.vector.scalar_tensor_tensor(
                    out=ob[:], in0=o_t[:], scalar=rec[:], in1=b2r_sb[:],
                    op0=ALU.mult, op1=ALU.add)
                r2 = wp.tile([128, HID2], F32, tag="r2")
                nc.scalar.activation(r2[:], ob[:], AF.Relu)
                nc.tensor.matmul(
                    pool_ps[:], lhsT=P_sb[:, b * G:(b + 1) * G], rhs=r2[:],
                    start=(b == 0), stop=(b == PB - 1))

            # each core returns its pooled partial sums; the host adds the
            # 8 partials and applies the tiny [64,2] head
            pooled = wp.tile([G, HID2], F32, tag="pool")
            nc.vector.tensor_copy(pooled[:], pool_ps[:])
            nc.sync.dma_start(out=out_d[:], in_=pooled[:])
            if debug_outs:
                nc.sync.dma_start(out=dbg_t1[:], in_=T1full[:])
                nc.sync.dma_start(out=dbg_sd1[:], in_=sd1[:])

    nc.compile()
    return nc


# ----------------------------------------------------------------------------
# Entry point
# ----------------------------------------------------------------------------

LAST_RESULTS = None
LAST_BUILD = None


def kernel(**inputs):
    global LAST_RESULTS, LAST_BUILD
    cfg = full_cfg()
    in_maps, meta = preprocess(cfg=cfg, **inputs)
    nc = build_program(cfg, meta)
    LAST_BUILD = (nc, in_maps, meta)
    res = run_bass_kernel_spmd(nc, in_maps, core_ids=list(range(cfg["NCORES"])))
    LAST_RESULTS = res
    pooled = np.sum([np.asarray(r["out"], np.float32) for r in res.results],
                    axis=0)
    WlBl = meta["WlBl"]
    return (pooled @ WlBl[:-1] + WlBl[-1]).astype(np.float32)
